# revision 1
# baseline (speedup 1.0000x reference)
"""CoordEncoder Trainium2 kernel, v2.

Data-parallel over B across 8 NeuronCores (one batch element per core).
Per core, for its L=1024 atoms (i-tiles of 128 rows, j free over all 1024):
  q[i,j] = |x_i - x_j|^2 via one K=5 augmented matmul on PE (aug matrices
  packed on host).
  d = sqrt(q + eps) in one ACT op (reads PSUM directly).
  16 RBF planes f_r = exp(-gamma*(d - c_r)^2) in bf16:
    - seed r=0:  exp(-gamma*q) on ACT straight from PSUM
    - seed r=6:  Square(d - c6) + Exp on ACT
    - seed r=12: (q - 2*c12*d) on DVE scalar_tensor_tensor, Exp on ACT
    - the rest chained with one bf16 tensor_tensor mult per plane
      (f~_r = f~_{r-1} * ts, ts = exp(2*gamma*dc*d + TSB)); the resulting
      constant per-plane drift is divided out of Wg on the host.
      Chains run on DVE (2x bf16) except GPLANES which go to GpSimd.
  Reductions over the neighbor axis j:
    - ACT-seeded planes: free accum_out row-sums on the ACT op -> rs[128,16]
    - DVE_RED planes: 4x-mode tensor_scalar pass with accum_out -> rs
    - all other planes: PE ones-column matmuls (col sums == row sums by
      symmetry) accumulated over i-tiles into a persistent cs[16,1024] PSUM
  rs is transposed per-tile on PE (identity matmul); the bf16 tail per tile
  is three N=256 matmuls into one PSUM: onehot(Z)^T @ t1 + rs^T @ Wg +
  cs-slice @ Wg.  Every plane's sum lives in exactly one of rs/cs (the
  other holds zeros), so the two Wg matmuls sum to the exact total.

Host side only packs layouts and folds input-independent weights.
"""

import numpy as np

B, L, E, R, NA = 8, 1024, 256, 16, 118
P = 128          # partition tile
NT = L // P      # 8 i-tiles per core
SEEDS = (0, 6, 12)      # planes seeded directly on ACT
GPLANES = (5, 11, 15)   # chained planes computed on GpSimd instead of DVE
DVE_RED = ()            # chained planes reduced via DVE tensor_scalar pass
Q_F32R = False          # q matmul in float32r (full-rate) instead of fp32
TSB = -5.0           # shared bias inside ts = exp(2*gamma*dc*d + TSB)
EPS = 1e-4           # sqrt(q + EPS); covers fp32 matmul cancellation error

_CACHE = {}


def _seg_of(r):
    s = max(x for x in SEEDS if x <= r)
    return s, r - s


def _build_nc(gamma, centers, split=True):
    import concourse.bass as bass
    import concourse.tile as tile
    from concourse import mybir
    from contextlib import ExitStack

    f32 = mybir.dt.float32
    bf16 = mybir.dt.bfloat16
    AF = mybir.ActivationFunctionType
    ALU = mybir.AluOpType

    dc = float(centers[1] - centers[0])
    c6, c12 = float(centers[6]), float(centers[12])
    PERED = [r for r in range(R) if r not in SEEDS and r not in DVE_RED]

    nc = bass.Bass("TRN2", target_bir_lowering=False, debug=False)

    lhsa = nc.dram_tensor("lhsa", [5, L], f32, kind="ExternalInput")
    rhsa = nc.dram_tensor("rhsa", [5, L], f32, kind="ExternalInput")
    t1 = nc.dram_tensor("t1", [NA, E], bf16, kind="ExternalInput")
    wg = nc.dram_tensor("wg", [R, E], bf16, kind="ExternalInput")
    oneh = nc.dram_tensor("oneh", [NA, L], bf16, kind="ExternalInput")
    ident = nc.dram_tensor("ident", [P, P], f32, kind="ExternalInput")
    onesc = nc.dram_tensor("onesc", [P, R * R], bf16, kind="ExternalInput")
    out = nc.dram_tensor("out", [L, E], f32, kind="ExternalOutput")

    with tile.TileContext(nc) as tc, ExitStack() as ctx:
        consts = ctx.enter_context(tc.tile_pool(name="consts", bufs=1))

        # ---- persistent SBUF tensors (all DMA'd straight from host) ----
        lhs_aug = consts.tile([5, L], f32)      # [-2x,-2y,-2z, 1, |x|^2]
        rhs_aug = consts.tile([5, L], f32)      # [x, y, z, |x|^2, 1]
        t1s = consts.tile([NA, E], bf16)
        wgs = consts.tile([R, E], bf16)
        onehotT = consts.tile([NA, L], bf16)    # onehot(Z)^T
        idn = consts.tile([P, P], f32)
        onescol = consts.tile([P, R, R], bf16)  # one-hot cols for PE colsums
        nc.sync.dma_start(lhs_aug[:], lhsa[:, :])
        nc.scalar.dma_start(rhs_aug[:], rhsa[:, :])
        nc.scalar.dma_start(onescol[:], onesc[:, :])
        nc.sync.dma_start(idn[:], ident[:, :])
        nc.scalar.dma_start(wgs[:], wg[:, :])
        nc.sync.dma_start(t1s[:], t1[:, :])
        nc.sync.dma_start(onehotT[:], oneh[:, :])

        # per-partition bias constants for activation ops
        eps_b = consts.tile([P, 1], f32, tag="eps_b")
        nc.vector.memset(eps_b[:], EPS)
        tsb = consts.tile([P, 1], f32, tag="tsb")
        nc.vector.memset(tsb[:], TSB)
        nc6 = consts.tile([P, 1], f32, tag="nc6")
        nc.vector.memset(nc6[:], -c6)
        g12b = consts.tile([P, 1], f32, tag="g12b")
        nc.vector.memset(g12b[:], -gamma * c12 * c12)

        # ---- pools ----
        from contextlib import ExitStack as _ES
        loop_ctx = _ES()
        cspp = ctx.enter_context(tc.tile_pool(name="cs_ps", bufs=1, space="PSUM"))
        qpp = loop_ctx.enter_context(tc.tile_pool(name="q_ps", bufs=2, space="PSUM"))
        rtpp = loop_ctx.enter_context(tc.tile_pool(name="rt_ps", bufs=2, space="PSUM"))
        ddp = ctx.enter_context(tc.tile_pool(name="dd", bufs=3))
        ttp = ctx.enter_context(tc.tile_pool(name="tt", bufs=3))
        sqp = ctx.enter_context(tc.tile_pool(name="sq", bufs=3))
        fbp = ctx.enter_context(tc.tile_pool(name="fb", bufs=20))
        trp = ctx.enter_context(tc.tile_pool(name="tr", bufs=2))
        rsp = ctx.enter_context(tc.tile_pool(name="rs", bufs=4))
        rtsp = ctx.enter_context(tc.tile_pool(name="rts", bufs=1))
        csbp = ctx.enter_context(tc.tile_pool(name="csb", bufs=1))
        hbp = ctx.enter_context(tc.tile_pool(name="hb", bufs=3))

        if Q_F32R:
            f32r = mybir.dt.float32r
            lhs_r = consts.tile([5, L], f32r, tag="lhs_r")
            rhs_r = consts.tile([5, L], f32r, tag="rhs_r")
            nc.vector.tensor_copy(lhs_r[:], lhs_aug[:])
            nc.vector.tensor_copy(rhs_r[:], rhs_aug[:])
            lhs_q, rhs_q = lhs_r, rhs_r
        else:
            lhs_q, rhs_q = lhs_aug, rhs_aug

        cs = cspp.tile([R, L], f32)     # PE-reduced plane sums (col sums)
        rsT = []                        # per-tile transposed row sums (SBUF)

        # PE warmup: ramp the tensor engine during the input DMAs so the
        # first real matmuls run at full clock (cold-PE fp32 is ~3x slower).
        wz = consts.tile([1, 512], bf16, tag="wz")
        nc.vector.memset(wz[:], 0.0)
        for w in range(6):
            nc.tensor.matmul(cs[0:1, 0:512], wz[0:1, 0:1], wz[:, :],
                             start=True, stop=True, skip_group_check=True)

        n_colsum_total = NT * len(PERED)
        colsum_state = {"n": 0}
        colsum_q = []

        def emit_colsums():
            while colsum_q:
                r, f = colsum_q.pop(0)
                for h in range(2):
                    nc.tensor.matmul(cs[:, h * 512:(h + 1) * 512],
                                     onescol[:, r, :],
                                     f[:, h * 512:(h + 1) * 512],
                                     start=(colsum_state["n"] == 0),
                                     stop=(colsum_state["n"] == n_colsum_total - 1 and h == 1),
                                     skip_group_check=True)
                colsum_state["n"] += 1

        for it in range(NT):
            i0 = it * P
            # pairwise squared distances for this row block: [128, 1024]
            qps = qpp.tile([P, L], f32)
            for h in range(2):
                nc.tensor.matmul(qps[:, h * 512:(h + 1) * 512],
                                 lhs_q[:, i0:i0 + P],
                                 rhs_q[:, h * 512:(h + 1) * 512],
                                 start=True, stop=True)
            emit_colsums()
            rs = rsp.tile([P, R], f32)
            nc.vector.memset(rs[:], 0.0)

            dd = ddp.tile([P, L], f32)
            nc.scalar.activation(dd[:], qps[:], AF.Sqrt, bias=eps_b[:])
            ts = ttp.tile([P, L], bf16)
            nc.scalar.activation(ts[:], dd[:], AF.Exp,
                                 scale=2.0 * gamma * dc, bias=tsb[:])

            planes = {}
            # seed 0: exp(-gamma*q) straight from PSUM
            f0 = fbp.tile([P, L], bf16, tag="plane")
            nc.scalar.activation(f0[:], qps[:], AF.Exp, scale=-gamma,
                                 accum_out=rs[:, 0:1])
            planes[0] = f0
            # seed 6: Square(d - c6) then Exp
            sq6 = sqp.tile([P, L], f32)
            nc.scalar.activation(sq6[:], dd[:], AF.Square, bias=nc6[:])
            f6 = fbp.tile([P, L], bf16, tag="plane")
            nc.scalar.activation(f6[:], sq6[:], AF.Exp, scale=-gamma,
                                 accum_out=rs[:, 6:7])
            planes[6] = f6
            # seed 12: (q - 2*c12*d) on DVE, then Exp(-gamma*x - gamma*c12^2)
            sh12 = sqp.tile([P, L], f32, tag="sh12")
            nc.vector.scalar_tensor_tensor(sh12[:], dd[:], -2.0 * c12, qps[:],
                                           op0=ALU.mult, op1=ALU.add)
            f12 = fbp.tile([P, L], bf16, tag="plane")
            nc.scalar.activation(f12[:], sh12[:], AF.Exp, scale=-gamma,
                                 bias=g12b[:], accum_out=rs[:, 12:13])
            planes[12] = f12

            # chains
            for s in SEEDS:
                f = planes[s]
                end = min(s + 6, R)
                for r in range(s + 1, end):
                    fn = fbp.tile([P, L], bf16, tag="plane")
                    # last tile: keep the slow GpSimd chains off the drain path
                    eng = nc.gpsimd if (r in GPLANES and it < NT - 1) else nc.vector
                    eng.tensor_tensor(fn[:], f[:], ts[:], ALU.mult)
                    if r in DVE_RED:
                        trash = trp.tile([P, L], bf16)
                        nc.vector.tensor_scalar(trash[:], fn[:], 1.0, None,
                                                ALU.mult, accum_out=rs[:, r:r + 1])
                    planes[r] = fn
                    f = fn

            # reductions: DVE 4x pass for DVE_RED planes
            for r in DVE_RED:
                trash = trp.tile([P, L], bf16)
                nc.vector.tensor_scalar(trash[:], planes[r][:], 1.0, None,
                                        ALU.mult, accum_out=rs[:, r:r + 1])
            # transpose rs early (depends only on the ACT seed accums) so it
            # slips into a PE bubble and unblocks rs buffer reuse
            rtp = rtpp.tile([R, P], f32)
            nc.tensor.transpose(rtp[:], rs[:], idn[:])
            rts = rtsp.tile([R, P], bf16, tag=f"rts{it}")
            nc.scalar.copy(rts[:], rtp[:])
            rsT.append(rts)
            # PE ones-column col sums; pool-produced planes finish ~2us after
            # the DVE chains, so defer their col sums one tile (emitted after
            # the next tile's q matmul) to keep PE from stalling
            for r in PERED:
                if not (r in GPLANES and it < NT - 1):
                    colsum_q.append((r, planes[r]))
            emit_colsums()
            for r in PERED:
                if r in GPLANES and it < NT - 1:
                    colsum_q.append((r, planes[r]))

        emit_colsums()

        # ---- tail: project and emit ----
        loop_ctx.close()
        csb = csbp.tile([R, L], bf16)
        with tc.tile_pool(name="h_ps", bufs=2, space="PSUM") as hpp:
            for it in range(NT):
                i0 = it * P
                nc.vector.tensor_copy(csb[:, i0:i0 + P], cs[:, i0:i0 + P])
                hps = hpp.tile([P, E], f32)
                nc.tensor.matmul(hps[:], onehotT[:, i0:i0 + P], t1s[:],
                                 start=True, stop=False, skip_group_check=True)
                nc.tensor.matmul(hps[:], rsT[it][:], wgs[:],
                                 start=False, stop=False, skip_group_check=True)
                nc.tensor.matmul(hps[:], csb[:, i0:i0 + P], wgs[:],
                                 start=False, stop=True, skip_group_check=True)
                hb = hbp.tile([P, E], f32)
                nc.scalar.copy(hb[:], hps[:])
                nc.sync.dma_start(out[i0:i0 + P, :], hb[:])

    if split:
        _split_excess_waits(nc)
    return nc


def _split_excess_waits(nc, maxw=1):
    """This walrus build rejects instructions carrying more than one sem wait
    (setupSyncWait: 'Too many sync wait commands'). Move excess waits onto
    injected same-engine NOPs that execute immediately before."""
    from concourse import mybir
    n = 0
    for fn in nc.m.functions:
        for bb in fn.blocks:
            new = []
            for ins in bb.instructions:
                si = ins.sync_info
                if si is not None and si.on_wait and len(si.on_wait) > maxw:
                    waits = list(si.on_wait)
                    excess, keep = waits[:-maxw], waits[-maxw:]
                    for ci in range(0, len(excess), maxw):
                        nop = mybir.InstNoOp(name=f"waitsplit_{ins.name}_{ci}",
                                             ins=[], outs=[])
                        nop.engine = ins.engine
                        nop.bass_nofuse = True
                        nop.sync_info = mybir.SyncInfo(on_wait=excess[ci:ci + maxw],
                                                       on_update=[])
                        new.append(nop)
                        n += 1
                    si.on_wait = keep
                new.append(ins)
            bb.instructions[:] = new
    return n


def _np_bf16():
    from concourse import mybir
    return mybir.dt.np(mybir.dt.bfloat16)


def _prep_inputs(coords, Z, atom_emb, rbf_centers, gamma, rbf_proj_w,
                 rbf_proj_b, out_proj_w, out_proj_b):
    f64 = np.float64
    bf = _np_bf16()
    g = float(np.asarray(gamma))
    centers = np.asarray(rbf_centers, dtype=f64)
    dc = float(centers[1] - centers[0])
    w1 = np.asarray(out_proj_w)[:E].astype(f64)
    w2 = np.asarray(out_proj_w)[E:].astype(f64)
    bias = (np.asarray(rbf_proj_b).astype(f64) @ w2) + np.asarray(out_proj_b).astype(f64)
    t1 = (np.asarray(atom_emb).astype(f64) @ w1 + bias).astype(bf)
    wgm = (np.asarray(rbf_proj_w).astype(f64) @ w2) / L
    # divide out the chain drift from using the shared ts (see _build_nc)
    for r in range(R):
        s, m = _seg_of(r)
        if m:
            wgm[r] /= np.exp(m * TSB + g * dc * (2 * centers[s] * m + dc * m * m))
    wgm = wgm.astype(bf)
    ident = np.eye(P, dtype=np.float32)
    onesc = np.zeros((P, R * R), dtype=np.float64)
    for r in range(R):
        onesc[:, r * R + r] = 1.0
    onesc = onesc.astype(bf)
    cf = np.asarray(coords, dtype=f64)                     # [B, L, 3]
    nsq = (cf * cf).sum(-1)                                # [B, L]
    ones = np.ones((L,), dtype=f64)
    Zl = np.asarray(Z)
    in_maps = []
    for b in range(B):
        xT = cf[b].T                                       # [3, L]
        lhs = np.concatenate([-2.0 * xT, ones[None, :], nsq[b][None, :]], axis=0)
        rhs = np.concatenate([xT, nsq[b][None, :], ones[None, :]], axis=0)
        onehotT = np.zeros((NA, L), dtype=np.float64)
        onehotT[Zl[b], np.arange(L)] = 1.0
        in_maps.append({
            "lhsa": lhs.astype(np.float32),
            "rhsa": rhs.astype(np.float32),
            "t1": t1, "wg": wgm,
            "oneh": onehotT.astype(bf),
            "ident": ident, "onesc": onesc,
        })
    return in_maps


def _get_nc(gamma, centers):
    key = (float(gamma),) + tuple(float(c) for c in centers)
    if key not in _CACHE:
        _CACHE[key] = _build_nc(float(gamma), [float(c) for c in centers])
    return _CACHE[key]


def _run(in_maps, gamma, centers, trace=False):
    from concourse.bass_utils import run_bass_kernel_spmd
    nc = _get_nc(gamma, centers)
    return run_bass_kernel_spmd(nc, in_maps, core_ids=list(range(B)), trace=trace)


def kernel(coords, Z, atom_emb, rbf_centers, gamma, rbf_proj_w, rbf_proj_b,
           out_proj_w, out_proj_b):
    centers = np.asarray(rbf_centers, dtype=np.float64)
    steps = np.diff(centers)
    assert np.allclose(steps, steps[0], rtol=1e-5), "uniform RBF grid expected"
    in_maps = _prep_inputs(coords, Z, atom_emb, rbf_centers, gamma, rbf_proj_w,
                           rbf_proj_b, out_proj_w, out_proj_b)
    res = _run(in_maps, float(np.asarray(gamma)), centers)
    return np.stack([res.results[b]["out"] for b in range(B)], axis=0)



# revision 4
# speedup vs baseline: 4.2702x; 4.2702x over previous
"""CoordEncoder Trainium2 kernel, v3.

v2 computed everything in ~100us of device time per core, but the end-to-end
execute rate through the axon PJRT tunnel is dominated by a per-operand,
per-execute dispatch cost (~1.5-3.5 ms per ExternalInput/Output tensor) plus
a per-device cost (~0.3 ms/core), NOT by bytes moved (+4 MB costs ~0.1 ms).
v3 therefore restructures the host interface:

  - ALL host inputs are packed into ONE f32 DRAM tensor `pk`; ONE output.
  - Runs on a single NeuronCore (N_CORES=1), looping over all B=8 batch
    elements on device; ~0.7 ms of device time hides under the ~4 ms
    per-execute dispatch floor of the tunnel.
  - Everything v2 shipped as separate tensors is built on device instead:
    identity matrices via gpsimd.affine_select, per-plane ones-columns via
    memsets, and the atom one-hot from packed f32 Z columns via
    iota + is_equal + a PE transpose.

Per batch the algorithm is v2's: q[i,j] = |x_i - x_j|^2 via one K=5
augmented matmul (aug matrices packed on host); d = sqrt(q + eps) on ACT;
16 RBF planes f_r = exp(-gamma*(d - c_r)^2) in bf16 from three ACT-seeded
planes (r = 0, 6, 12) and chained bf16 multiplies f_r = f_{r-1} * ts
(ts = exp(2*gamma*dc*d + TSB), constant drift divided out of Wg on host)
on DVE/GpSimd; plane sums over the neighbor axis via free ACT accum_out
(seeds -> rs, transposed per-tile on PE) and PE ones-column colsums
(chains -> cs; colsum == rowsum by symmetry of f). Tail per 128-row tile:
h = onehot(Z)^T t1 + rs^T Wg + cs Wg, with t1/Wg folded on host.
"""

import numpy as np

B, L, E, R, NA = 8, 1024, 256, 16, 118
P = 128          # partition tile
NT = L // P      # 8 i-tiles per batch element
N_CORES = 1      # cores used; each runs B // N_CORES batch elements
NB = B // N_CORES
SEEDS = (0, 6, 12)      # planes seeded directly on ACT
GPLANES = (5, 11, 15)   # chained planes computed on GpSimd instead of DVE
TSB = -5.0           # shared bias inside ts = exp(2*gamma*dc*d + TSB)
EPS = 1e-4           # sqrt(q + EPS); covers fp32 matmul cancellation error

# pk layout (per core), f32 [128 + 10*NB, 1024]:
#   rows 0:128, cols 0:NB*NT          Z columns: col b*NT+it = Z[b, it*P:(it+1)*P]
#   rows 0:118, cols CT1:CT1+256      t1 (atom_emb @ W1 + bias, f32)
#   rows 0:16,  cols CWG:CWG+256      wg (rbf_proj_w @ W2 / L, drift-folded)
#   rows 128+10*b : +5                lhs aug rows [-2x,-2y,-2z, 1, |x|^2]
#   rows 128+10*b+5 : +10             rhs aug rows [x, y, z, |x|^2, 1]
ZC = NB * NT
CT1 = ZC
CWG = CT1 + 256
PK_ROWS = 128 + 10 * NB

_CACHE = {}


def _seg_of(r):
    s = max(x for x in SEEDS if x <= r)
    return s, r - s


def _build_nc(gamma, centers, split=True):
    import concourse.bass as bass
    import concourse.tile as tile
    from concourse import mybir
    from concourse.masks import make_identity
    from contextlib import ExitStack

    f32 = mybir.dt.float32
    bf16 = mybir.dt.bfloat16
    i32 = mybir.dt.int32
    AF = mybir.ActivationFunctionType
    ALU = mybir.AluOpType

    dc = float(centers[1] - centers[0])
    c6, c12 = float(centers[6]), float(centers[12])
    PERED = [r for r in range(R) if r not in SEEDS]

    nc = bass.Bass("TRN2", target_bir_lowering=False, debug=False)

    pk = nc.dram_tensor("pk", [PK_ROWS, 1024], f32, kind="ExternalInput")
    out = nc.dram_tensor("out", [NB * L, E], f32, kind="ExternalOutput")

    with tile.TileContext(nc) as tc, ExitStack() as ctx:
        consts = ctx.enter_context(tc.tile_pool(name="consts", bufs=1))

        # ---- one DMA for the whole constant block ----
        blk = consts.tile([P, CWG + 256], f32)
        nc.sync.dma_start(blk[:], pk[0:P, 0:CWG + 256])
        t1s = consts.tile([NA, E], bf16)
        nc.scalar.copy(t1s[:], blk[0:NA, CT1:CT1 + 256])
        wgs = consts.tile([R, E], bf16)
        nc.scalar.copy(wgs[:], blk[0:R, CWG:CWG + 256])

        # ---- on-device constants ----
        iota_i = consts.tile([P, NA], i32)
        nc.gpsimd.iota(iota_i[:], pattern=[[1, NA]], base=0, channel_multiplier=0)
        iota_f = consts.tile([P, NA], f32)
        nc.vector.tensor_copy(iota_f[:], iota_i[:])
        idn = consts.tile([P, P], f32)
        make_identity(nc, idn[:])
        onescol = consts.tile([P, R, R], bf16)  # one-hot cols for PE colsums
        nc.vector.memset(onescol[:], 0.0)
        for r in range(R):
            nc.vector.memset(onescol[:, r, r:r + 1], 1.0)

        # per-partition bias constants for activation ops
        eps_b = consts.tile([P, 1], f32, tag="eps_b")
        nc.vector.memset(eps_b[:], EPS)
        tsb = consts.tile([P, 1], f32, tag="tsb")
        nc.vector.memset(tsb[:], TSB)
        nc6 = consts.tile([P, 1], f32, tag="nc6")
        nc.vector.memset(nc6[:], -c6)
        g12b = consts.tile([P, 1], f32, tag="g12b")
        nc.vector.memset(g12b[:], -gamma * c12 * c12)

        # persistent SBUF accumulators consumed by the deferred tail
        onehT_all = consts.tile([NA, NB * L], bf16)
        csb_all = consts.tile([R, NB * L], bf16)

        # ---- pools ----
        from contextlib import ExitStack as _ES
        loop_ctx = _ES()
        cspp = loop_ctx.enter_context(tc.tile_pool(name="cs_ps", bufs=1, space="PSUM"))
        qpp = loop_ctx.enter_context(tc.tile_pool(name="q_ps", bufs=2, space="PSUM"))
        trp = loop_ctx.enter_context(tc.tile_pool(name="tr_ps", bufs=2, space="PSUM"))
        lrp = ctx.enter_context(tc.tile_pool(name="lr", bufs=2))
        ddp = ctx.enter_context(tc.tile_pool(name="dd", bufs=3))
        ttp = ctx.enter_context(tc.tile_pool(name="tt", bufs=3))
        sqp = ctx.enter_context(tc.tile_pool(name="sq", bufs=3))
        fbp = ctx.enter_context(tc.tile_pool(name="fb", bufs=20))
        ohp = ctx.enter_context(tc.tile_pool(name="oh", bufs=3))
        rsp = ctx.enter_context(tc.tile_pool(name="rs", bufs=4))
        rtsp = ctx.enter_context(tc.tile_pool(name="rts", bufs=1))
        hbp = ctx.enter_context(tc.tile_pool(name="hb", bufs=3))

        # PE warmup: ramp the tensor engine during the input DMA so the
        # first real matmuls run at full clock (cold-PE fp32 is ~3x slower).
        wz = consts.tile([1, 512], bf16, tag="wz")
        nc.vector.memset(wz[:], 0.0)

        n_colsum_total = NT * len(PERED)
        rsT = []

        for b in range(NB):
            roff = P + 10 * b
            lhs_aug = lrp.tile([5, L], f32, tag="lhs")
            nc.sync.dma_start(lhs_aug[:], pk[roff:roff + 5, :])
            rhs_aug = lrp.tile([5, L], f32, tag="rhs")
            nc.scalar.dma_start(rhs_aug[:], pk[roff + 5:roff + 10, :])

            cs = cspp.tile([R, L], f32)     # PE-reduced plane sums (col sums)
            if b == 0:
                for w in range(6):
                    nc.tensor.matmul(cs[0:1, 0:512], wz[0:1, 0:1], wz[:, :],
                                     start=True, stop=True, skip_group_check=True)

            colsum_state = {"n": 0}
            colsum_q = []

            def emit_colsums(cs=cs, colsum_state=colsum_state, colsum_q=colsum_q):
                while colsum_q:
                    r, f = colsum_q.pop(0)
                    for h in range(2):
                        nc.tensor.matmul(cs[:, h * 512:(h + 1) * 512],
                                         onescol[:, r, :],
                                         f[:, h * 512:(h + 1) * 512],
                                         start=(colsum_state["n"] == 0),
                                         stop=(colsum_state["n"] == n_colsum_total - 1 and h == 1),
                                         skip_group_check=True)
                    colsum_state["n"] += 1

            for it in range(NT):
                i0 = it * P
                # pairwise squared distances for this row block: [128, 1024]
                qps = qpp.tile([P, L], f32)
                for h in range(2):
                    nc.tensor.matmul(qps[:, h * 512:(h + 1) * 512],
                                     lhs_aug[:, i0:i0 + P],
                                     rhs_aug[:, h * 512:(h + 1) * 512],
                                     start=True, stop=True)
                emit_colsums()

                # atom one-hot for this tile: oh[j, a] = (Z[j] == a), then
                # PE-transpose into onehT_all[a, j]
                zc = b * NT + it
                oh = ohp.tile([P, NA], f32)
                nc.vector.tensor_scalar(oh[:], iota_f[:], blk[:, zc:zc + 1], None,
                                        ALU.is_equal)
                ohT = trp.tile([NA, P], f32, tag="tr")
                nc.tensor.transpose(ohT[:], oh[:], idn[:])
                nc.scalar.copy(onehT_all[:, b * L + i0:b * L + i0 + P], ohT[:])

                rs = rsp.tile([P, R], f32)
                nc.vector.memset(rs[:], 0.0)

                dd = ddp.tile([P, L], f32)
                nc.scalar.activation(dd[:], qps[:], AF.Sqrt, bias=eps_b[:])
                ts = ttp.tile([P, L], bf16)
                nc.scalar.activation(ts[:], dd[:], AF.Exp,
                                     scale=2.0 * gamma * dc, bias=tsb[:])

                planes = {}
                # seed 0: exp(-gamma*q) straight from PSUM
                f0 = fbp.tile([P, L], bf16, tag="plane")
                nc.scalar.activation(f0[:], qps[:], AF.Exp, scale=-gamma,
                                     accum_out=rs[:, 0:1])
                planes[0] = f0
                # seed 6: Square(d - c6) then Exp
                sq6 = sqp.tile([P, L], f32)
                nc.scalar.activation(sq6[:], dd[:], AF.Square, bias=nc6[:])
                f6 = fbp.tile([P, L], bf16, tag="plane")
                nc.scalar.activation(f6[:], sq6[:], AF.Exp, scale=-gamma,
                                     accum_out=rs[:, 6:7])
                planes[6] = f6
                # seed 12: (q - 2*c12*d) on DVE, then Exp(-gamma*x - gamma*c12^2)
                sh12 = sqp.tile([P, L], f32, tag="sh12")
                nc.vector.scalar_tensor_tensor(sh12[:], dd[:], -2.0 * c12, qps[:],
                                               op0=ALU.mult, op1=ALU.add)
                f12 = fbp.tile([P, L], bf16, tag="plane")
                nc.scalar.activation(f12[:], sh12[:], AF.Exp, scale=-gamma,
                                     bias=g12b[:], accum_out=rs[:, 12:13])
                planes[12] = f12

                # chains
                for s in SEEDS:
                    f = planes[s]
                    end = min(s + 6, R)
                    for r in range(s + 1, end):
                        fn = fbp.tile([P, L], bf16, tag="plane")
                        # last tile: keep the slow GpSimd chains off the drain path
                        eng = nc.gpsimd if (r in GPLANES and it < NT - 1) else nc.vector
                        eng.tensor_tensor(fn[:], f[:], ts[:], ALU.mult)
                        planes[r] = fn
                        f = fn

                # transpose rs early (depends only on the ACT seed accums) so it
                # slips into a PE bubble and unblocks rs buffer reuse
                rtp = trp.tile([NA, P], f32, tag="tr")
                nc.tensor.transpose(rtp[0:R, :], rs[:], idn[:])
                rts = rtsp.tile([R, P], bf16, tag=f"rts{b}_{it}")
                nc.scalar.copy(rts[:], rtp[0:R, :])
                rsT.append(rts)
                # PE ones-column col sums; pool-produced planes finish ~2us after
                # the DVE chains, so defer their col sums one tile (emitted after
                # the next tile's q matmul) to keep PE from stalling
                for r in PERED:
                    if not (r in GPLANES and it < NT - 1):
                        colsum_q.append((r, planes[r]))
                emit_colsums()
                for r in PERED:
                    if r in GPLANES and it < NT - 1:
                        colsum_q.append((r, planes[r]))

            emit_colsums()
            nc.vector.tensor_copy(csb_all[:, b * L:(b + 1) * L], cs[:])

        # ---- tail: project and emit ----
        loop_ctx.close()
        with tc.tile_pool(name="h_ps", bufs=4, space="PSUM") as hpp:
            for b in range(NB):
                for it in range(NT):
                    i0 = b * L + it * P
                    hps = hpp.tile([P, E], f32)
                    nc.tensor.matmul(hps[:], onehT_all[:, i0:i0 + P], t1s[:],
                                     start=True, stop=False, skip_group_check=True)
                    nc.tensor.matmul(hps[:], rsT[b * NT + it][:], wgs[:],
                                     start=False, stop=False, skip_group_check=True)
                    nc.tensor.matmul(hps[:], csb_all[:, i0:i0 + P], wgs[:],
                                     start=False, stop=True, skip_group_check=True)
                    hb = hbp.tile([P, E], f32)
                    nc.scalar.copy(hb[:], hps[:])
                    nc.sync.dma_start(out[i0:i0 + P, :], hb[:])

    if split:
        _split_excess_waits(nc)
    return nc


def _split_excess_waits(nc, maxw=1):
    """This walrus build rejects instructions carrying more than one sem wait
    (setupSyncWait: 'Too many sync wait commands'). Move excess waits onto
    injected same-engine NOPs that execute immediately before."""
    from concourse import mybir
    n = 0
    for fn in nc.m.functions:
        for bb in fn.blocks:
            new = []
            for ins in bb.instructions:
                si = ins.sync_info
                if si is not None and si.on_wait and len(si.on_wait) > maxw:
                    waits = list(si.on_wait)
                    excess, keep = waits[:-maxw], waits[-maxw:]
                    for ci in range(0, len(excess), maxw):
                        nop = mybir.InstNoOp(name=f"waitsplit_{ins.name}_{ci}",
                                             ins=[], outs=[])
                        nop.engine = ins.engine
                        nop.bass_nofuse = True
                        nop.sync_info = mybir.SyncInfo(on_wait=excess[ci:ci + maxw],
                                                       on_update=[])
                        new.append(nop)
                        n += 1
                    si.on_wait = keep
                new.append(ins)
            bb.instructions[:] = new
    return n


def _prep_inputs(coords, Z, atom_emb, rbf_centers, gamma, rbf_proj_w,
                 rbf_proj_b, out_proj_w, out_proj_b):
    f64 = np.float64
    g = float(np.asarray(gamma))
    centers = np.asarray(rbf_centers, dtype=f64)
    dc = float(centers[1] - centers[0])
    w1 = np.asarray(out_proj_w)[:E].astype(f64)
    w2 = np.asarray(out_proj_w)[E:].astype(f64)
    bias = (np.asarray(rbf_proj_b).astype(f64) @ w2) + np.asarray(out_proj_b).astype(f64)
    t1 = (np.asarray(atom_emb).astype(f64) @ w1 + bias).astype(np.float32)
    wgm = (np.asarray(rbf_proj_w).astype(f64) @ w2) / L
    # divide out the chain drift from using the shared ts (see _build_nc)
    for r in range(R):
        s, m = _seg_of(r)
        if m:
            wgm[r] /= np.exp(m * TSB + g * dc * (2 * centers[s] * m + dc * m * m))
    wgm = wgm.astype(np.float32)
    cf = np.asarray(coords, dtype=f64)                     # [B, L, 3]
    nsq = (cf * cf).sum(-1)                                # [B, L]
    ones = np.ones((L,), dtype=f64)
    Zl = np.asarray(Z)
    in_maps = []
    for c in range(N_CORES):
        pk = np.zeros((PK_ROWS, 1024), dtype=np.float32)
        pk[0:NA, CT1:CT1 + 256] = t1
        pk[0:R, CWG:CWG + 256] = wgm
        for lb in range(NB):
            gb = c * NB + lb
            pk[0:P, lb * NT:(lb + 1) * NT] = Zl[gb].reshape(NT, P).T.astype(np.float32)
            xT = cf[gb].T                                  # [3, L]
            roff = P + 10 * lb
            pk[roff:roff + 3, :] = (-2.0 * xT).astype(np.float32)
            pk[roff + 3, :] = 1.0
            pk[roff + 4, :] = nsq[gb].astype(np.float32)
            pk[roff + 5:roff + 8, :] = xT.astype(np.float32)
            pk[roff + 8, :] = nsq[gb].astype(np.float32)
            pk[roff + 9, :] = 1.0
        in_maps.append({"pk": pk})
    return in_maps


def _get_nc(gamma, centers):
    key = (float(gamma),) + tuple(float(c) for c in centers)
    if key not in _CACHE:
        _CACHE[key] = _build_nc(float(gamma), [float(c) for c in centers])
    return _CACHE[key]


def _run(in_maps, gamma, centers, trace=False):
    from concourse.bass_utils import run_bass_kernel_spmd
    nc = _get_nc(gamma, centers)
    return run_bass_kernel_spmd(nc, in_maps, core_ids=list(range(N_CORES)),
                                trace=trace)


def kernel(coords, Z, atom_emb, rbf_centers, gamma, rbf_proj_w, rbf_proj_b,
           out_proj_w, out_proj_b):
    centers = np.asarray(rbf_centers, dtype=np.float64)
    steps = np.diff(centers)
    assert np.allclose(steps, steps[0], rtol=1e-5), "uniform RBF grid expected"
    in_maps = _prep_inputs(coords, Z, atom_emb, rbf_centers, gamma, rbf_proj_w,
                           rbf_proj_b, out_proj_w, out_proj_b)
    res = _run(in_maps, float(np.asarray(gamma)), centers)
    return np.concatenate(
        [res.results[c]["out"].reshape(NB, L, E) for c in range(N_CORES)],
        axis=0)


# revision 7
# speedup vs baseline: 5.0511x; 1.1829x over previous
"""CoordEncoder Trainium2 kernel, v3.

v2 computed everything in ~100us of device time per core, but the end-to-end
execute rate through the axon PJRT tunnel is dominated by a per-operand,
per-execute dispatch cost (~1.5-3.5 ms per ExternalInput/Output tensor) plus
a per-device cost (~0.3 ms/core), NOT by bytes moved (+4 MB costs ~0.1 ms).
v3 therefore restructures the host interface:

  - ALL host inputs are packed into ONE f32 DRAM tensor `pk`; ONE output.
  - Runs on a single NeuronCore (N_CORES=1), looping over all B=8 batch
    elements on device; ~0.7 ms of device time hides under the ~4 ms
    per-execute dispatch floor of the tunnel.
  - Everything v2 shipped as separate tensors is built on device instead:
    identity matrices via gpsimd.affine_select, per-plane ones-columns via
    memsets, and the atom one-hot from packed f32 Z columns via
    iota + is_equal + a PE transpose.

Per batch the algorithm is v2's: q[i,j] = |x_i - x_j|^2 via one K=5
augmented matmul (aug matrices packed on host); d = sqrt(q + eps) on ACT;
16 RBF planes f_r = exp(-gamma*(d - c_r)^2) in bf16 from three ACT-seeded
planes (r = 0, 6, 12) and chained bf16 multiplies f_r = f_{r-1} * ts
(ts = exp(2*gamma*dc*d + TSB), constant drift divided out of Wg on host)
on DVE/GpSimd; plane sums over the neighbor axis via free ACT accum_out
(seeds -> rs, transposed per-tile on PE) and PE ones-column colsums
(chains -> cs; colsum == rowsum by symmetry of f). Tail per 128-row tile:
h = onehot(Z)^T t1 + rs^T Wg + cs Wg, with t1/Wg folded on host.
"""

import numpy as np

B, L, E, R, NA = 8, 1024, 256, 16, 118
P = 128          # partition tile
NT = L // P      # 8 i-tiles per batch element
N_CORES = 1      # cores used; each runs B // N_CORES batch elements
NB = B // N_CORES
SEEDS = (0, 6, 12)      # planes seeded directly on ACT
GPLANES = (5, 11, 15)   # chained planes computed on GpSimd instead of DVE
TSB = -5.0           # shared bias inside ts = exp(2*gamma*dc*d + TSB)
EPS = 1e-4           # sqrt(q + EPS); covers fp32 matmul cancellation error

# pk layout (per core), f32 [128 + 10*NB, 1024]:
#   rows 0:128, cols 0:NB*NT          Z columns: col b*NT+it = Z[b, it*P:(it+1)*P]
#   rows 0:118, cols CT1:CT1+256      t1 (atom_emb @ W1 + bias, f32)
#   rows 0:16,  cols CWG:CWG+256      wg (rbf_proj_w @ W2 / L, drift-folded)
#   rows 128+10*b : +5                lhs aug rows [-2x,-2y,-2z, 1, |x|^2]
#   rows 128+10*b+5 : +10             rhs aug rows [x, y, z, |x|^2, 1]
ZC = NB * NT
CT1 = ZC
CWG = CT1 + 256
PK_ROWS = 128 + 10 * NB

_CACHE = {}


def _seg_of(r):
    s = max(x for x in SEEDS if x <= r)
    return s, r - s


def _build_nc(gamma, centers, split=True):
    import concourse.bass as bass
    import concourse.tile as tile
    from concourse import mybir
    from concourse.masks import make_identity
    from contextlib import ExitStack

    f32 = mybir.dt.float32
    bf16 = mybir.dt.bfloat16
    i32 = mybir.dt.int32
    AF = mybir.ActivationFunctionType
    ALU = mybir.AluOpType

    dc = float(centers[1] - centers[0])
    c6, c12 = float(centers[6]), float(centers[12])
    PERED = [r for r in range(R) if r not in SEEDS]

    nc = bass.Bass("TRN2", target_bir_lowering=False, debug=False)

    pk = nc.dram_tensor("pk", [PK_ROWS, 1024], f32, kind="ExternalInput")
    out = nc.dram_tensor("out", [NB * L, E], bf16, kind="ExternalOutput")

    with tile.TileContext(nc) as tc, ExitStack() as ctx:
        consts = ctx.enter_context(tc.tile_pool(name="consts", bufs=1))

        # ---- one DMA for the whole constant block ----
        blk = consts.tile([P, CWG + 256], f32)
        nc.sync.dma_start(blk[:], pk[0:P, 0:CWG + 256])
        t1s = consts.tile([NA, E], bf16)
        nc.scalar.copy(t1s[:], blk[0:NA, CT1:CT1 + 256])
        wgs = consts.tile([R, E], bf16)
        nc.scalar.copy(wgs[:], blk[0:R, CWG:CWG + 256])

        # ---- on-device constants ----
        iota_i = consts.tile([P, NA], i32)
        nc.gpsimd.iota(iota_i[:], pattern=[[1, NA]], base=0, channel_multiplier=0)
        iota_f = consts.tile([P, NA], f32)
        nc.vector.tensor_copy(iota_f[:], iota_i[:])
        idn = consts.tile([P, P], f32)
        make_identity(nc, idn[:])
        onescol = consts.tile([P, R, R], bf16)  # one-hot cols for PE colsums
        nc.vector.memset(onescol[:], 0.0)
        for r in range(R):
            nc.vector.memset(onescol[:, r, r:r + 1], 1.0)

        # per-partition bias constants for activation ops
        eps_b = consts.tile([P, 1], f32, tag="eps_b")
        nc.vector.memset(eps_b[:], EPS)
        tsb = consts.tile([P, 1], f32, tag="tsb")
        nc.vector.memset(tsb[:], TSB)
        nc6 = consts.tile([P, 1], f32, tag="nc6")
        nc.vector.memset(nc6[:], -c6)
        g12b = consts.tile([P, 1], f32, tag="g12b")
        nc.vector.memset(g12b[:], -gamma * c12 * c12)

        # persistent SBUF accumulators consumed by the deferred tail
        onehT_all = consts.tile([NA, NB * L], bf16)
        csb_all = consts.tile([R, NB * L], bf16)

        # ---- pools ----
        from contextlib import ExitStack as _ES
        loop_ctx = _ES()
        cspp = loop_ctx.enter_context(tc.tile_pool(name="cs_ps", bufs=1, space="PSUM"))
        qpp = loop_ctx.enter_context(tc.tile_pool(name="q_ps", bufs=2, space="PSUM"))
        trp = loop_ctx.enter_context(tc.tile_pool(name="tr_ps", bufs=2, space="PSUM"))
        lrp = ctx.enter_context(tc.tile_pool(name="lr", bufs=2))
        ddp = ctx.enter_context(tc.tile_pool(name="dd", bufs=3))
        ttp = ctx.enter_context(tc.tile_pool(name="tt", bufs=3))
        sqp = ctx.enter_context(tc.tile_pool(name="sq", bufs=3))
        fbp = ctx.enter_context(tc.tile_pool(name="fb", bufs=20))
        ohp = ctx.enter_context(tc.tile_pool(name="oh", bufs=3))
        rsp = ctx.enter_context(tc.tile_pool(name="rs", bufs=4))
        rtsp = ctx.enter_context(tc.tile_pool(name="rts", bufs=1))
        hbp = ctx.enter_context(tc.tile_pool(name="hb", bufs=3))

        # PE warmup: ramp the tensor engine during the input DMA so the
        # first real matmuls run at full clock (cold-PE fp32 is ~3x slower).
        wz = consts.tile([1, 512], bf16, tag="wz")
        nc.vector.memset(wz[:], 0.0)

        n_colsum_total = NT * len(PERED)
        rsT = []

        for b in range(NB):
            roff = P + 10 * b
            lhs_aug = lrp.tile([5, L], f32, tag="lhs")
            nc.sync.dma_start(lhs_aug[:], pk[roff:roff + 5, :])
            rhs_aug = lrp.tile([5, L], f32, tag="rhs")
            nc.scalar.dma_start(rhs_aug[:], pk[roff + 5:roff + 10, :])

            cs = cspp.tile([R, L], f32)     # PE-reduced plane sums (col sums)
            if b == 0:
                for w in range(6):
                    nc.tensor.matmul(cs[0:1, 0:512], wz[0:1, 0:1], wz[:, :],
                                     start=True, stop=True, skip_group_check=True)

            colsum_state = {"n": 0}
            colsum_q = []

            def emit_colsums(cs=cs, colsum_state=colsum_state, colsum_q=colsum_q):
                while colsum_q:
                    r, f = colsum_q.pop(0)
                    for h in range(2):
                        nc.tensor.matmul(cs[:, h * 512:(h + 1) * 512],
                                         onescol[:, r, :],
                                         f[:, h * 512:(h + 1) * 512],
                                         start=(colsum_state["n"] == 0),
                                         stop=(colsum_state["n"] == n_colsum_total - 1 and h == 1),
                                         skip_group_check=True)
                    colsum_state["n"] += 1

            for it in range(NT):
                i0 = it * P
                # pairwise squared distances for this row block: [128, 1024]
                qps = qpp.tile([P, L], f32)
                for h in range(2):
                    nc.tensor.matmul(qps[:, h * 512:(h + 1) * 512],
                                     lhs_aug[:, i0:i0 + P],
                                     rhs_aug[:, h * 512:(h + 1) * 512],
                                     start=True, stop=True)
                emit_colsums()

                # atom one-hot for this tile: oh[j, a] = (Z[j] == a), then
                # PE-transpose into onehT_all[a, j]
                zc = b * NT + it
                oh = ohp.tile([P, NA], f32)
                nc.vector.tensor_scalar(oh[:], iota_f[:], blk[:, zc:zc + 1], None,
                                        ALU.is_equal)
                ohT = trp.tile([NA, P], f32, tag="tr")
                nc.tensor.transpose(ohT[:], oh[:], idn[:])
                nc.scalar.copy(onehT_all[:, b * L + i0:b * L + i0 + P], ohT[:])

                rs = rsp.tile([P, R], f32)
                nc.vector.memset(rs[:], 0.0)

                dd = ddp.tile([P, L], f32)
                nc.scalar.activation(dd[:], qps[:], AF.Sqrt, bias=eps_b[:])
                ts = ttp.tile([P, L], bf16)
                nc.scalar.activation(ts[:], dd[:], AF.Exp,
                                     scale=2.0 * gamma * dc, bias=tsb[:])

                planes = {}
                # seed 0: exp(-gamma*q) straight from PSUM
                f0 = fbp.tile([P, L], bf16, tag="plane")
                nc.scalar.activation(f0[:], qps[:], AF.Exp, scale=-gamma,
                                     accum_out=rs[:, 0:1])
                planes[0] = f0
                # seed 6: Square(d - c6) then Exp
                sq6 = sqp.tile([P, L], f32)
                nc.scalar.activation(sq6[:], dd[:], AF.Square, bias=nc6[:])
                f6 = fbp.tile([P, L], bf16, tag="plane")
                nc.scalar.activation(f6[:], sq6[:], AF.Exp, scale=-gamma,
                                     accum_out=rs[:, 6:7])
                planes[6] = f6
                # seed 12: (q - 2*c12*d) on DVE, then Exp(-gamma*x - gamma*c12^2)
                sh12 = sqp.tile([P, L], f32, tag="sh12")
                nc.vector.scalar_tensor_tensor(sh12[:], dd[:], -2.0 * c12, qps[:],
                                               op0=ALU.mult, op1=ALU.add)
                f12 = fbp.tile([P, L], bf16, tag="plane")
                nc.scalar.activation(f12[:], sh12[:], AF.Exp, scale=-gamma,
                                     bias=g12b[:], accum_out=rs[:, 12:13])
                planes[12] = f12

                # chains
                for s in SEEDS:
                    f = planes[s]
                    end = min(s + 6, R)
                    for r in range(s + 1, end):
                        fn = fbp.tile([P, L], bf16, tag="plane")
                        # last tile: keep the slow GpSimd chains off the drain path
                        eng = nc.gpsimd if (r in GPLANES and it < NT - 1) else nc.vector
                        eng.tensor_tensor(fn[:], f[:], ts[:], ALU.mult)
                        planes[r] = fn
                        f = fn

                # transpose rs early (depends only on the ACT seed accums) so it
                # slips into a PE bubble and unblocks rs buffer reuse
                rtp = trp.tile([NA, P], f32, tag="tr")
                nc.tensor.transpose(rtp[0:R, :], rs[:], idn[:])
                rts = rtsp.tile([R, P], bf16, tag=f"rts{b}_{it}")
                nc.scalar.copy(rts[:], rtp[0:R, :])
                rsT.append(rts)
                # PE ones-column col sums; pool-produced planes finish ~2us after
                # the DVE chains, so defer their col sums one tile (emitted after
                # the next tile's q matmul) to keep PE from stalling
                for r in PERED:
                    if not (r in GPLANES and it < NT - 1):
                        colsum_q.append((r, planes[r]))
                emit_colsums()
                for r in PERED:
                    if r in GPLANES and it < NT - 1:
                        colsum_q.append((r, planes[r]))

            emit_colsums()
            nc.vector.tensor_copy(csb_all[:, b * L:(b + 1) * L], cs[:])

        # ---- tail: project and emit ----
        loop_ctx.close()
        with tc.tile_pool(name="h_ps", bufs=4, space="PSUM") as hpp:
            for b in range(NB):
                for it in range(NT):
                    i0 = b * L + it * P
                    hps = hpp.tile([P, E], f32)
                    nc.tensor.matmul(hps[:], onehT_all[:, i0:i0 + P], t1s[:],
                                     start=True, stop=False, skip_group_check=True)
                    nc.tensor.matmul(hps[:], rsT[b * NT + it][:], wgs[:],
                                     start=False, stop=False, skip_group_check=True)
                    nc.tensor.matmul(hps[:], csb_all[:, i0:i0 + P], wgs[:],
                                     start=False, stop=True, skip_group_check=True)
                    hb = hbp.tile([P, E], bf16)
                    nc.scalar.copy(hb[:], hps[:])
                    nc.sync.dma_start(out[i0:i0 + P, :], hb[:])

    if split:
        _split_excess_waits(nc)
    return nc


def _split_excess_waits(nc, maxw=1):
    """This walrus build rejects instructions carrying more than one sem wait
    (setupSyncWait: 'Too many sync wait commands'). Move excess waits onto
    injected same-engine NOPs that execute immediately before."""
    from concourse import mybir
    n = 0
    for fn in nc.m.functions:
        for bb in fn.blocks:
            new = []
            for ins in bb.instructions:
                si = ins.sync_info
                if si is not None and si.on_wait and len(si.on_wait) > maxw:
                    waits = list(si.on_wait)
                    excess, keep = waits[:-maxw], waits[-maxw:]
                    for ci in range(0, len(excess), maxw):
                        nop = mybir.InstNoOp(name=f"waitsplit_{ins.name}_{ci}",
                                             ins=[], outs=[])
                        nop.engine = ins.engine
                        nop.bass_nofuse = True
                        nop.sync_info = mybir.SyncInfo(on_wait=excess[ci:ci + maxw],
                                                       on_update=[])
                        new.append(nop)
                        n += 1
                    si.on_wait = keep
                new.append(ins)
            bb.instructions[:] = new
    return n


def _prep_inputs(coords, Z, atom_emb, rbf_centers, gamma, rbf_proj_w,
                 rbf_proj_b, out_proj_w, out_proj_b):
    f64 = np.float64
    g = float(np.asarray(gamma))
    centers = np.asarray(rbf_centers, dtype=f64)
    dc = float(centers[1] - centers[0])
    w1 = np.asarray(out_proj_w)[:E].astype(f64)
    w2 = np.asarray(out_proj_w)[E:].astype(f64)
    bias = (np.asarray(rbf_proj_b).astype(f64) @ w2) + np.asarray(out_proj_b).astype(f64)
    t1 = (np.asarray(atom_emb).astype(f64) @ w1 + bias).astype(np.float32)
    wgm = (np.asarray(rbf_proj_w).astype(f64) @ w2) / L
    # divide out the chain drift from using the shared ts (see _build_nc)
    for r in range(R):
        s, m = _seg_of(r)
        if m:
            wgm[r] /= np.exp(m * TSB + g * dc * (2 * centers[s] * m + dc * m * m))
    wgm = wgm.astype(np.float32)
    cf = np.asarray(coords, dtype=f64)                     # [B, L, 3]
    nsq = (cf * cf).sum(-1)                                # [B, L]
    ones = np.ones((L,), dtype=f64)
    Zl = np.asarray(Z)
    in_maps = []
    for c in range(N_CORES):
        pk = np.zeros((PK_ROWS, 1024), dtype=np.float32)
        pk[0:NA, CT1:CT1 + 256] = t1
        pk[0:R, CWG:CWG + 256] = wgm
        for lb in range(NB):
            gb = c * NB + lb
            pk[0:P, lb * NT:(lb + 1) * NT] = Zl[gb].reshape(NT, P).T.astype(np.float32)
            xT = cf[gb].T                                  # [3, L]
            roff = P + 10 * lb
            pk[roff:roff + 3, :] = (-2.0 * xT).astype(np.float32)
            pk[roff + 3, :] = 1.0
            pk[roff + 4, :] = nsq[gb].astype(np.float32)
            pk[roff + 5:roff + 8, :] = xT.astype(np.float32)
            pk[roff + 8, :] = nsq[gb].astype(np.float32)
            pk[roff + 9, :] = 1.0
        in_maps.append({"pk": pk})
    return in_maps


def _get_nc(gamma, centers):
    key = (float(gamma),) + tuple(float(c) for c in centers)
    if key not in _CACHE:
        _CACHE[key] = _build_nc(float(gamma), [float(c) for c in centers])
    return _CACHE[key]


def _get_exec(gamma, centers):
    """Build (once) and cache a jitted PJRT callable for the Bass module.
    Returns (fn, in_names, out_names, zero_outs). run_bass_kernel_spmd /
    run_bass_via_pjrt re-trace and re-jit on every call, which costs seconds;
    this is the same execution path with the callable kept alive."""
    import jax
    from concourse import bass2jax, mybir

    key = ("exec", float(gamma)) + tuple(float(c) for c in centers)
    if key in _CACHE:
        return _CACHE[key]
    nc = _get_nc(gamma, centers)
    bass2jax.install_neuronx_cc_hook()
    pname = nc.partition_id_tensor.name if nc.partition_id_tensor else None
    in_names, out_names, out_avals, zero_outs = [], [], [], []
    for alloc in nc.m.functions[0].allocations:
        if not isinstance(alloc, mybir.MemoryLocationSet):
            continue
        name = alloc.memorylocations[0].name
        if alloc.kind == "ExternalInput":
            if name != pname:
                in_names.append(name)
        elif alloc.kind == "ExternalOutput":
            out_names.append(name)
            shape = tuple(alloc.tensor_shape)
            dtype = mybir.dt.np(alloc.dtype)
            out_avals.append(jax.core.ShapedArray(shape, dtype))
            zero_outs.append(np.zeros(shape, dtype))
    all_names = in_names + out_names
    if pname is not None:
        all_names = all_names + [pname]

    def _body(*args):
        operands = list(args)
        if pname is not None:
            operands.append(bass2jax.partition_id_tensor())
        outs = bass2jax._bass_exec_p.bind(
            *operands, out_avals=tuple(out_avals), in_names=tuple(all_names),
            out_names=tuple(out_names), lowering_input_output_aliases=(),
            sim_require_finite=True, sim_require_nnan=True, nc=nc)
        return tuple(outs)

    fn = jax.jit(_body)
    _CACHE[key] = (fn, in_names, out_names, zero_outs)
    return _CACHE[key]


def _run(in_maps, gamma, centers):
    import jax
    fn, in_names, out_names, zero_outs = _get_exec(gamma, centers)
    dev = jax.devices()[0]
    args = [jax.device_put(np.asarray(in_maps[0][n]), dev) for n in in_names]
    args += [jax.device_put(z, dev) for z in zero_outs]
    outs = fn(*args)
    return {n: np.asarray(o) for n, o in zip(out_names, outs)}


def kernel(coords, Z, atom_emb, rbf_centers, gamma, rbf_proj_w, rbf_proj_b,
           out_proj_w, out_proj_b):
    centers = np.asarray(rbf_centers, dtype=np.float64)
    steps = np.diff(centers)
    assert np.allclose(steps, steps[0], rtol=1e-5), "uniform RBF grid expected"
    in_maps = _prep_inputs(coords, Z, atom_emb, rbf_centers, gamma, rbf_proj_w,
                           rbf_proj_b, out_proj_w, out_proj_b)
    res = _run(in_maps, float(np.asarray(gamma)), centers)
    return res["out"].reshape(B, L, E).astype(np.float32)


# revision 10
# speedup vs baseline: 5.7752x; 1.1433x over previous
"""CoordEncoder Trainium2 kernel, v3.

v2 computed everything in ~100us of device time per core, but the end-to-end
execute rate through the axon PJRT tunnel is dominated by a per-operand,
per-execute dispatch cost (~1.5-3.5 ms per ExternalInput/Output tensor) plus
a per-device cost (~0.3 ms/core), NOT by bytes moved (+4 MB costs ~0.1 ms).
v3 therefore restructures the host interface:

  - ALL host inputs are packed into ONE f32 DRAM tensor `pk`; ONE output.
  - Runs on a single NeuronCore (N_CORES=1), looping over all B=8 batch
    elements on device; ~0.7 ms of device time hides under the ~4 ms
    per-execute dispatch floor of the tunnel.
  - Everything v2 shipped as separate tensors is built on device instead:
    identity matrices via gpsimd.affine_select, per-plane ones-columns via
    memsets, and the atom one-hot from packed f32 Z columns via
    iota + is_equal + a PE transpose.

Per batch the algorithm is v2's: q[i,j] = |x_i - x_j|^2 via one K=5
augmented matmul (aug matrices packed on host); d = sqrt(q + eps) on ACT;
16 RBF planes f_r = exp(-gamma*(d - c_r)^2) in bf16 from three ACT-seeded
planes (r = 0, 6, 12) and chained bf16 multiplies f_r = f_{r-1} * ts
(ts = exp(2*gamma*dc*d + TSB), constant drift divided out of Wg on host)
on DVE/GpSimd; plane sums over the neighbor axis via free ACT accum_out
(seeds -> rs, transposed per-tile on PE) and PE ones-column colsums
(chains -> cs; colsum == rowsum by symmetry of f). Tail per 128-row tile:
h = onehot(Z)^T t1 + rs^T Wg + cs Wg, with t1/Wg folded on host.
"""

import numpy as np

B, L, E, R, NA = 8, 1024, 256, 16, 118
P = 128          # partition tile
NT = L // P      # 8 i-tiles per batch element
N_CORES = 1      # cores used; each runs B // N_CORES batch elements
NB = B // N_CORES
SEEDS = (0, 6, 12)      # planes seeded directly on ACT
GPLANES = (5, 11, 15)   # chained planes computed on GpSimd instead of DVE
TSB = -5.0           # shared bias inside ts = exp(2*gamma*dc*d + TSB)
EPS = 1e-4           # sqrt(q + EPS); covers fp32 matmul cancellation error

# pk layout (per core), f32 [128 + 10*NB, 1024]:
#   rows 0:128, cols 0:NB*NT          Z columns: col b*NT+it = Z[b, it*P:(it+1)*P]
#   rows 0:118, cols CT1:CT1+256      t1 (atom_emb @ W1 + bias, f32)
#   rows 0:16,  cols CWG:CWG+256      wg (rbf_proj_w @ W2 / L, drift-folded)
#   rows 128+10*b : +5                lhs aug rows [-2x,-2y,-2z, 1, |x|^2]
#   rows 128+10*b+5 : +10             rhs aug rows [x, y, z, |x|^2, 1]
ZC = NB * NT
CT1 = ZC
CWG = CT1 + 256
PK_ROWS = 128 + 10 * NB

_CACHE = {}


def _seg_of(r):
    s = max(x for x in SEEDS if x <= r)
    return s, r - s


def _build_nc(gamma, centers, split=True):
    import concourse.bass as bass
    import concourse.tile as tile
    from concourse import mybir
    from concourse.masks import make_identity
    from contextlib import ExitStack

    f32 = mybir.dt.float32
    bf16 = mybir.dt.bfloat16
    i32 = mybir.dt.int32
    AF = mybir.ActivationFunctionType
    ALU = mybir.AluOpType

    dc = float(centers[1] - centers[0])
    c6, c12 = float(centers[6]), float(centers[12])
    PERED = [r for r in range(R) if r not in SEEDS]

    nc = bass.Bass("TRN2", target_bir_lowering=False, debug=False)

    pk = nc.dram_tensor("pk", [PK_ROWS, 1024], f32, kind="ExternalInput")
    out = nc.dram_tensor("out", [NB * L, E], bf16, kind="ExternalOutput")

    with tile.TileContext(nc) as tc, ExitStack() as ctx:
        consts = ctx.enter_context(tc.tile_pool(name="consts", bufs=1))

        # ---- one DMA for the whole constant block ----
        blk = consts.tile([P, CWG + 256], f32)
        nc.sync.dma_start(blk[:], pk[0:P, 0:CWG + 256])
        t1s = consts.tile([NA, E], bf16)
        nc.scalar.copy(t1s[:], blk[0:NA, CT1:CT1 + 256])
        wgs = consts.tile([R, E], bf16)
        nc.scalar.copy(wgs[:], blk[0:R, CWG:CWG + 256])

        # ---- on-device constants ----
        iota_i = consts.tile([P, NA], i32)
        nc.gpsimd.iota(iota_i[:], pattern=[[1, NA]], base=0, channel_multiplier=0)
        iota_f = consts.tile([P, NA], f32)
        nc.vector.tensor_copy(iota_f[:], iota_i[:])
        idn = consts.tile([P, P], f32)
        make_identity(nc, idn[:])
        onescol = consts.tile([P, R, R], bf16)  # one-hot cols for PE colsums
        nc.vector.memset(onescol[:], 0.0)
        for r in range(R):
            nc.vector.memset(onescol[:, r, r:r + 1], 1.0)

        # per-partition bias constants for activation ops
        eps_b = consts.tile([P, 1], f32, tag="eps_b")
        nc.vector.memset(eps_b[:], EPS)
        tsb = consts.tile([P, 1], f32, tag="tsb")
        nc.vector.memset(tsb[:], TSB)
        nc6 = consts.tile([P, 1], f32, tag="nc6")
        nc.vector.memset(nc6[:], -c6)
        g12b = consts.tile([P, 1], f32, tag="g12b")
        nc.vector.memset(g12b[:], -gamma * c12 * c12)

        # persistent SBUF accumulators consumed by the deferred tail
        onehT_all = consts.tile([NA, NB * L], bf16)
        csb_all = consts.tile([R, NB * L], bf16)

        # ---- pools ----
        from contextlib import ExitStack as _ES
        loop_ctx = _ES()
        cspp = loop_ctx.enter_context(tc.tile_pool(name="cs_ps", bufs=1, space="PSUM"))
        qpp = loop_ctx.enter_context(tc.tile_pool(name="q_ps", bufs=2, space="PSUM"))
        trp = loop_ctx.enter_context(tc.tile_pool(name="tr_ps", bufs=2, space="PSUM"))
        lrp = ctx.enter_context(tc.tile_pool(name="lr", bufs=2))
        ddp = ctx.enter_context(tc.tile_pool(name="dd", bufs=3))
        ttp = ctx.enter_context(tc.tile_pool(name="tt", bufs=3))
        sqp = ctx.enter_context(tc.tile_pool(name="sq", bufs=3))
        fbp = ctx.enter_context(tc.tile_pool(name="fb", bufs=20))
        ohp = ctx.enter_context(tc.tile_pool(name="oh", bufs=3))
        rsp = ctx.enter_context(tc.tile_pool(name="rs", bufs=4))
        rtsp = ctx.enter_context(tc.tile_pool(name="rts", bufs=1))
        hbp = ctx.enter_context(tc.tile_pool(name="hb", bufs=3))

        # PE warmup: ramp the tensor engine during the input DMA so the
        # first real matmuls run at full clock (cold-PE fp32 is ~3x slower).
        wz = consts.tile([1, 512], bf16, tag="wz")
        nc.vector.memset(wz[:], 0.0)

        n_colsum_total = NT * len(PERED)
        rsT = []

        for b in range(NB):
            roff = P + 10 * b
            lhs_aug = lrp.tile([5, L], f32, tag="lhs")
            nc.sync.dma_start(lhs_aug[:], pk[roff:roff + 5, :])
            rhs_aug = lrp.tile([5, L], f32, tag="rhs")
            nc.scalar.dma_start(rhs_aug[:], pk[roff + 5:roff + 10, :])

            cs = cspp.tile([R, L], f32)     # PE-reduced plane sums (col sums)
            if b == 0:
                for w in range(6):
                    nc.tensor.matmul(cs[0:1, 0:512], wz[0:1, 0:1], wz[:, :],
                                     start=True, stop=True, skip_group_check=True)

            colsum_state = {"n": 0}
            colsum_q = []

            def emit_colsums(cs=cs, colsum_state=colsum_state, colsum_q=colsum_q):
                while colsum_q:
                    r, f = colsum_q.pop(0)
                    for h in range(2):
                        nc.tensor.matmul(cs[:, h * 512:(h + 1) * 512],
                                         onescol[:, r, :],
                                         f[:, h * 512:(h + 1) * 512],
                                         start=(colsum_state["n"] == 0),
                                         stop=(colsum_state["n"] == n_colsum_total - 1 and h == 1),
                                         skip_group_check=True)
                    colsum_state["n"] += 1

            for it in range(NT):
                i0 = it * P
                # pairwise squared distances for this row block: [128, 1024]
                qps = qpp.tile([P, L], f32)
                for h in range(2):
                    nc.tensor.matmul(qps[:, h * 512:(h + 1) * 512],
                                     lhs_aug[:, i0:i0 + P],
                                     rhs_aug[:, h * 512:(h + 1) * 512],
                                     start=True, stop=True)
                emit_colsums()

                # atom one-hot for this tile: oh[j, a] = (Z[j] == a), then
                # PE-transpose into onehT_all[a, j]
                zc = b * NT + it
                oh = ohp.tile([P, NA], f32)
                nc.vector.tensor_scalar(oh[:], iota_f[:], blk[:, zc:zc + 1], None,
                                        ALU.is_equal)
                ohT = trp.tile([NA, P], f32, tag="tr")
                nc.tensor.transpose(ohT[:], oh[:], idn[:])
                nc.scalar.copy(onehT_all[:, b * L + i0:b * L + i0 + P], ohT[:])

                rs = rsp.tile([P, R], f32)
                nc.vector.memset(rs[:], 0.0)

                dd = ddp.tile([P, L], f32)
                nc.scalar.activation(dd[:], qps[:], AF.Sqrt, bias=eps_b[:])
                ts = ttp.tile([P, L], bf16)
                nc.scalar.activation(ts[:], dd[:], AF.Exp,
                                     scale=2.0 * gamma * dc, bias=tsb[:])

                planes = {}
                # seed 0: exp(-gamma*q) straight from PSUM
                f0 = fbp.tile([P, L], bf16, tag="plane")
                nc.scalar.activation(f0[:], qps[:], AF.Exp, scale=-gamma,
                                     accum_out=rs[:, 0:1])
                planes[0] = f0
                # seed 6: Square(d - c6) then Exp
                sq6 = sqp.tile([P, L], f32)
                nc.scalar.activation(sq6[:], dd[:], AF.Square, bias=nc6[:])
                f6 = fbp.tile([P, L], bf16, tag="plane")
                nc.scalar.activation(f6[:], sq6[:], AF.Exp, scale=-gamma,
                                     accum_out=rs[:, 6:7])
                planes[6] = f6
                # seed 12: (q - 2*c12*d) on DVE, then Exp(-gamma*x - gamma*c12^2)
                sh12 = sqp.tile([P, L], f32, tag="sh12")
                nc.vector.scalar_tensor_tensor(sh12[:], dd[:], -2.0 * c12, qps[:],
                                               op0=ALU.mult, op1=ALU.add)
                f12 = fbp.tile([P, L], bf16, tag="plane")
                nc.scalar.activation(f12[:], sh12[:], AF.Exp, scale=-gamma,
                                     bias=g12b[:], accum_out=rs[:, 12:13])
                planes[12] = f12

                # chains
                for s in SEEDS:
                    f = planes[s]
                    end = min(s + 6, R)
                    for r in range(s + 1, end):
                        fn = fbp.tile([P, L], bf16, tag="plane")
                        # last tile: keep the slow GpSimd chains off the drain path
                        eng = nc.gpsimd if (r in GPLANES and it < NT - 1) else nc.vector
                        eng.tensor_tensor(fn[:], f[:], ts[:], ALU.mult)
                        planes[r] = fn
                        f = fn

                # transpose rs early (depends only on the ACT seed accums) so it
                # slips into a PE bubble and unblocks rs buffer reuse
                rtp = trp.tile([NA, P], f32, tag="tr")
                nc.tensor.transpose(rtp[0:R, :], rs[:], idn[:])
                rts = rtsp.tile([R, P], bf16, tag=f"rts{b}_{it}")
                nc.scalar.copy(rts[:], rtp[0:R, :])
                rsT.append(rts)
                # PE ones-column col sums; pool-produced planes finish ~2us after
                # the DVE chains, so defer their col sums one tile (emitted after
                # the next tile's q matmul) to keep PE from stalling
                for r in PERED:
                    if not (r in GPLANES and it < NT - 1):
                        colsum_q.append((r, planes[r]))
                emit_colsums()
                for r in PERED:
                    if r in GPLANES and it < NT - 1:
                        colsum_q.append((r, planes[r]))

            emit_colsums()
            nc.vector.tensor_copy(csb_all[:, b * L:(b + 1) * L], cs[:])

        # ---- tail: project and emit ----
        loop_ctx.close()
        with tc.tile_pool(name="h_ps", bufs=4, space="PSUM") as hpp:
            for b in range(NB):
                for it in range(NT):
                    i0 = b * L + it * P
                    hps = hpp.tile([P, E], f32)
                    nc.tensor.matmul(hps[:], onehT_all[:, i0:i0 + P], t1s[:],
                                     start=True, stop=False, skip_group_check=True)
                    nc.tensor.matmul(hps[:], rsT[b * NT + it][:], wgs[:],
                                     start=False, stop=False, skip_group_check=True)
                    nc.tensor.matmul(hps[:], csb_all[:, i0:i0 + P], wgs[:],
                                     start=False, stop=True, skip_group_check=True)
                    hb = hbp.tile([P, E], bf16)
                    nc.scalar.copy(hb[:], hps[:])
                    nc.sync.dma_start(out[i0:i0 + P, :], hb[:])

    if split:
        _split_excess_waits(nc)
    return nc


def _split_excess_waits(nc, maxw=1):
    """This walrus build rejects instructions carrying more than one sem wait
    (setupSyncWait: 'Too many sync wait commands'). Move excess waits onto
    injected same-engine NOPs that execute immediately before."""
    from concourse import mybir
    n = 0
    for fn in nc.m.functions:
        for bb in fn.blocks:
            new = []
            for ins in bb.instructions:
                si = ins.sync_info
                if si is not None and si.on_wait and len(si.on_wait) > maxw:
                    waits = list(si.on_wait)
                    excess, keep = waits[:-maxw], waits[-maxw:]
                    for ci in range(0, len(excess), maxw):
                        nop = mybir.InstNoOp(name=f"waitsplit_{ins.name}_{ci}",
                                             ins=[], outs=[])
                        nop.engine = ins.engine
                        nop.bass_nofuse = True
                        nop.sync_info = mybir.SyncInfo(on_wait=excess[ci:ci + maxw],
                                                       on_update=[])
                        new.append(nop)
                        n += 1
                    si.on_wait = keep
                new.append(ins)
            bb.instructions[:] = new
    return n


def _prep_inputs(coords, Z, atom_emb, rbf_centers, gamma, rbf_proj_w,
                 rbf_proj_b, out_proj_w, out_proj_b):
    f64 = np.float64
    g = float(np.asarray(gamma))
    centers = np.asarray(rbf_centers, dtype=f64)
    dc = float(centers[1] - centers[0])
    w1 = np.asarray(out_proj_w)[:E].astype(f64)
    w2 = np.asarray(out_proj_w)[E:].astype(f64)
    bias = (np.asarray(rbf_proj_b).astype(f64) @ w2) + np.asarray(out_proj_b).astype(f64)
    t1 = (np.asarray(atom_emb).astype(f64) @ w1 + bias).astype(np.float32)
    wgm = (np.asarray(rbf_proj_w).astype(f64) @ w2) / L
    # divide out the chain drift from using the shared ts (see _build_nc)
    for r in range(R):
        s, m = _seg_of(r)
        if m:
            wgm[r] /= np.exp(m * TSB + g * dc * (2 * centers[s] * m + dc * m * m))
    wgm = wgm.astype(np.float32)
    cf = np.asarray(coords, dtype=f64)                     # [B, L, 3]
    nsq = (cf * cf).sum(-1)                                # [B, L]
    ones = np.ones((L,), dtype=f64)
    Zl = np.asarray(Z)
    in_maps = []
    for c in range(N_CORES):
        pk = np.zeros((PK_ROWS, 1024), dtype=np.float32)
        pk[0:NA, CT1:CT1 + 256] = t1
        pk[0:R, CWG:CWG + 256] = wgm
        for lb in range(NB):
            gb = c * NB + lb
            pk[0:P, lb * NT:(lb + 1) * NT] = Zl[gb].reshape(NT, P).T.astype(np.float32)
            xT = cf[gb].T                                  # [3, L]
            roff = P + 10 * lb
            pk[roff:roff + 3, :] = (-2.0 * xT).astype(np.float32)
            pk[roff + 3, :] = 1.0
            pk[roff + 4, :] = nsq[gb].astype(np.float32)
            pk[roff + 5:roff + 8, :] = xT.astype(np.float32)
            pk[roff + 8, :] = nsq[gb].astype(np.float32)
            pk[roff + 9, :] = 1.0
        in_maps.append({"pk": pk})
    return in_maps


def _get_nc(gamma, centers):
    key = (float(gamma),) + tuple(float(c) for c in centers)
    if key not in _CACHE:
        _CACHE[key] = _build_nc(float(gamma), [float(c) for c in centers])
    return _CACHE[key]


def _get_exec(gamma, centers):
    """Build (once) and cache a jitted PJRT callable for the Bass module.
    Returns (fn, in_names, out_names, zero_outs). run_bass_kernel_spmd /
    run_bass_via_pjrt re-trace and re-jit on every call, which costs seconds;
    this is the same execution path with the callable kept alive."""
    import jax
    from concourse import bass2jax, mybir

    key = ("exec", float(gamma)) + tuple(float(c) for c in centers)
    if key in _CACHE:
        return _CACHE[key]
    nc = _get_nc(gamma, centers)
    bass2jax.install_neuronx_cc_hook()
    pname = nc.partition_id_tensor.name if nc.partition_id_tensor else None
    in_names, out_names, out_avals = [], [], []
    for alloc in nc.m.functions[0].allocations:
        if not isinstance(alloc, mybir.MemoryLocationSet):
            continue
        name = alloc.memorylocations[0].name
        if alloc.kind == "ExternalInput":
            if name != pname:
                in_names.append(name)
        elif alloc.kind == "ExternalOutput":
            out_names.append(name)
            shape = tuple(alloc.tensor_shape)
            dtype = mybir.dt.np(alloc.dtype)
            out_avals.append(jax.core.ShapedArray(shape, dtype))
    # The kernel writes every output element, so outputs are NOT threaded
    # through as donated zero-init operands (run_bass_via_pjrt does that for
    # kernels that leave elements unwritten). One fewer operand per execute
    # saves ~0.4 ms of axon per-operand dispatch cost.
    all_names = in_names + ([pname] if pname is not None else [])

    def _body(*args):
        operands = list(args)
        if pname is not None:
            operands.append(bass2jax.partition_id_tensor())
        outs = bass2jax._bass_exec_p.bind(
            *operands, out_avals=tuple(out_avals), in_names=tuple(all_names),
            out_names=tuple(out_names), lowering_input_output_aliases=(),
            sim_require_finite=True, sim_require_nnan=True, nc=nc)
        return tuple(outs)

    fn = jax.jit(_body)
    _CACHE[key] = (fn, in_names, out_names)
    return _CACHE[key]


def _run(in_maps, gamma, centers):
    import jax
    fn, in_names, out_names = _get_exec(gamma, centers)
    dev = jax.devices()[0]
    args = [jax.device_put(np.asarray(in_maps[0][n]), dev) for n in in_names]
    outs = fn(*args)
    return {n: np.asarray(o) for n, o in zip(out_names, outs)}


def kernel(coords, Z, atom_emb, rbf_centers, gamma, rbf_proj_w, rbf_proj_b,
           out_proj_w, out_proj_b):
    centers = np.asarray(rbf_centers, dtype=np.float64)
    steps = np.diff(centers)
    assert np.allclose(steps, steps[0], rtol=1e-5), "uniform RBF grid expected"
    in_maps = _prep_inputs(coords, Z, atom_emb, rbf_centers, gamma, rbf_proj_w,
                           rbf_proj_b, out_proj_w, out_proj_b)
    res = _run(in_maps, float(np.asarray(gamma)), centers)
    return res["out"].reshape(B, L, E).astype(np.float32)
